# revision 3
# baseline (speedup 1.0000x reference)
"""LGnet (LSTM-style gated recurrent net) Trainium2 Bass kernel.

B=64, T=256, D=512, H=512, COMB=1536. Data-parallel over batch across 8
NeuronCores (B_local=8 per core).

The axon tunnel to the cores moves ~38 MB/s H2D / ~22 MB/s D2H, so wall
time is transfer-bound, not compute-bound. Design, per call:
  - x ships per-core sliced, fp16, NATURAL layout ([8,4,256,512]); the
    feature-major transposes happen on device via XBAR DMA transpose.
  - the ~6.3M params ship fp16 ONCE in total: each core gets 1/8th
    (wpart) and an on-device AllGather reassembles the full pack.
  - output returns fp16 [4,128,8,256] per core.

Math decomposition (identical to the fp32 reference):
  comb @ W = xt' @ W[0:512] + h' @ W[512:1024] + m @ W[1024:1536].
  Only the h' term is recurrent; delta_x, delta_h, xt', and the
  non-recurrent gate preactivations "a" are precomputed for all (t,b)
  with large matmuls; the sequential phase does per-step [8,512] @
  [512,2048] fp16 matmuls against resident weights.

Per-core layouts (fp16 unless noted), free index r = b*256 + t:
  xtpT/mT : [128, 4, 8, 256]    x^T residents  [d%128, d//128, b, t]
  dht     : [128, 4, 8, 256]    delta_h^T
  aT      : [128, 16, 8, 256]   gate preactivations (i|f|o|c x4 chunks)
  wg      : [128, 8, 2048]      non-recurrent gate weights (xt rows then m rows)
  wh      : [128, 4, 2048]      recurrent rows 512:1024
  ghw     : [128, 4, 512]       gamma_h weight
  pvec    : [128, 28] f32       gxw(4) gxb(4) ghb(4) gate biases(16)
  h_out   : [4, 128, 8, 256]    h^T staged output [hc, p, b, t]
"""

import os
import numpy as np

B, T, D, H = 64, 256, 512, 512
NCORES = 8
BL = B // NCORES          # 8 per-core batch
DC, HC, GC = 4, 4, 16     # 128-chunks of D, H, and 4*H gate cols
G = 4 * H                 # 2048 gate columns
SS = 16                   # recurrence steps per output-staging block
WROWS = 1792              # weight pack rows (Wg 1024 + Wh 512 + ghW 128 + xm 128)
WSH = WROWS // NCORES     # 224 rows per core

_CACHE = {}


def _build():
    import concourse.bacc as bacc
    import concourse.tile as tile
    import concourse.mybir as mybir

    f32 = mybir.dt.float32
    f16 = mybir.dt.float16
    AF = mybir.ActivationFunctionType

    nc = bacc.Bacc("TRN2", target_bir_lowering=False, debug=False,
                   num_devices=NCORES)

    xc = nc.dram_tensor("xc", [BL, 4, T, D], f16, kind="ExternalInput").ap()
    wpart = nc.dram_tensor("wpart", [WSH, G], f16, kind="ExternalInput").ap()
    pv_in = nc.dram_tensor("pvec", [128, 28], f32, kind="ExternalInput").ap()
    out_ap = nc.dram_tensor("h_out", [HC, 128, BL, T], f16,
                            kind="ExternalOutput").ap()

    with tile.TileContext(nc) as tc:
        with (
            tc.tile_pool(name="const", bufs=1) as cp,
            tc.tile_pool(name="work", bufs=2) as wp,
            tc.tile_pool(name="xtr", bufs=2) as xp,
            tc.tile_pool(name="stage", bufs=2) as stp,
            tc.tile_pool(name="psA", bufs=2, space="PSUM") as ppa,
            tc.tile_pool(name="psB", bufs=2, space="PSUM") as ppb,
            tc.tile_pool(name="psR", bufs=2, space="PSUM") as ppr,
            tc.tile_pool(name="dram", bufs=1, space="DRAM") as dp,
        ):
            # ---- gather the weight pack from all cores ----
            # (collectives cannot read IO tensors -> bounce through internal DRAM)
            wbounce = dp.tile([WSH, G], f16)
            nc.sync.dma_start(wbounce[:], wpart[:])
            wfull = dp.tile([WROWS, G], f16)
            nc.gpsimd.collective_compute(
                "AllGather",
                mybir.AluOpType.bypass,
                replica_groups=[list(range(NCORES))],
                ins=[wbounce[:]],
                outs=[wfull[:]],
            )

            pvec = cp.tile([128, 28], f32)
            nc.sync.dma_start(pvec[:], pv_in[:])
            wg = cp.tile([128, 8, G], f16)
            for kc in range(8):
                nc.sync.dma_start(wg[:, kc, :], wfull[kc * 128:(kc + 1) * 128, :])
            wh = cp.tile([128, HC, G], f16)
            for kc in range(HC):
                nc.sync.dma_start(
                    wh[:, kc, :], wfull[1024 + kc * 128:1024 + (kc + 1) * 128, :])
            ghw = cp.tile([128, DC, D], f16)
            nc.sync.dma_start(ghw[:, :, :], wfull[1536:1664, :])
            xm = cp.tile([128, DC, T], f16)
            nc.sync.dma_start(xm[:, :, :], wfull[1664:1792, 0:DC * T])

            # ---- residents ----
            xtp = cp.tile([128, DC, BL, T], f16)   # xt^T, becomes xt'
            mT = cp.tile([128, DC, BL, T], f16)    # m^T
            dht = cp.tile([128, HC, BL, T], f16)   # delta_h^T
            aT = cp.tile([128, GC, BL, T], f16)    # gate preactivations

            # ---- phase 1 (per batch row): transpose + delta_h + delta_x + xt' ----
            for b in range(BL):
                dlT = xp.tile([128, DC, T], f16, tag="dlT")
                xlT = xp.tile([128, DC, T], f16, tag="xlT")
                nc.sync.dma_start_transpose(xtp[:, :, b, :], xc[b, 0])
                nc.sync.dma_start_transpose(xlT[:], xc[b, 1])
                nc.sync.dma_start_transpose(mT[:, :, b, :], xc[b, 2])
                nc.sync.dma_start_transpose(dlT[:], xc[b, 3])

                # delta_h = exp(-relu(dl @ ghW + ghb))
                for mt in range(HC):
                    ps = ppa.tile([128, T], f32, tag="dhps")
                    for kc in range(DC):
                        nc.tensor.matmul(
                            ps[:],
                            ghw[:, kc, mt * 128:(mt + 1) * 128],
                            dlT[:, kc, :],
                            start=(kc == 0), stop=(kc == DC - 1))
                    t1 = wp.tile([128, T], f32, tag="dht1")
                    nc.scalar.activation(t1[:], ps[:], AF.Relu,
                                         bias=pvec[:, 8 + mt:9 + mt], scale=1.0)
                    nc.scalar.activation(dht[:, mt, b, :], t1[:], AF.Exp,
                                         scale=-1.0)

                # delta_x = exp(-relu(dl * gxw + gxb)), in place into dlT
                for kc in range(DC):
                    t2 = wp.tile([128, T], f32, tag="dxt")
                    nc.scalar.activation(t2[:], dlT[:, kc, :], AF.Relu,
                                         bias=pvec[:, 4 + kc:5 + kc],
                                         scale=pvec[:, kc:kc + 1])
                    nc.scalar.activation(dlT[:, kc, :], t2[:], AF.Exp,
                                         scale=-1.0)

                # xt' = m*(xt - inner) + inner;  inner = dx*(xl - xm) + xm
                for kc in range(DC):
                    s1 = wp.tile([128, T], f16, tag="s1")
                    nc.vector.tensor_sub(s1[:], xlT[:, kc, :], xm[:, kc, :])
                    nc.vector.tensor_mul(s1[:], dlT[:, kc, :], s1[:])
                    nc.vector.tensor_add(s1[:], s1[:], xm[:, kc, :])
                    s2 = wp.tile([128, T], f16, tag="s2")
                    nc.vector.tensor_sub(s2[:], xtp[:, kc, b, :], s1[:])
                    nc.vector.tensor_mul(s2[:], mT[:, kc, b, :], s2[:])
                    nc.vector.tensor_add(xtp[:, kc, b, :], s2[:], s1[:])

            # ---- phase 2: a = xt'@Wx + m@Wm + bias ----
            for gc in range(GC):
                for nb in range(4):          # blocks of 2 batch rows = 512 cols
                    b0 = nb * 2
                    ps = ppb.tile([128, 2, T], f32, tag="aps")
                    for kc in range(8):
                        rhs = (xtp[:, kc, b0:b0 + 2, :] if kc < DC
                               else mT[:, kc - DC, b0:b0 + 2, :])
                        nc.tensor.matmul(ps[:], wg[:, kc, gc * 128:(gc + 1) * 128],
                                         rhs, start=(kc == 0), stop=(kc == 7))
                    nc.scalar.activation(aT[:, gc, b0:b0 + 2, :], ps[:],
                                         AF.Identity,
                                         bias=pvec[:, 12 + gc:13 + gc], scale=1.0)

            # ---- phase 3: recurrence ----
            c_st = cp.tile([128, HC, BL], f32)
            hbf = cp.tile([128, HC, BL], f16)
            nc.vector.memset(c_st[:], 0.0)
            nc.vector.memset(hbf[:], 0.0)

            for blk in range(T // SS):
                hst = stp.tile([128, HC, BL, SS], f16, tag="hst")
                for s in range(SS):
                    t = blk * SS + s
                    gps = ppr.tile([128, GC, BL], f32, tag="gps")
                    for gc in range(GC):
                        for kc in range(HC):
                            nc.tensor.matmul(
                                gps[:, gc, :],
                                wh[:, kc, gc * 128:(gc + 1) * 128],
                                hbf[:, kc, :],
                                start=(kc == 0), stop=(kc == HC - 1))
                    g = wp.tile([128, GC, BL], f32, tag="g")
                    nc.vector.tensor_copy(g[:], aT[:, :, :, t])
                    nc.vector.tensor_add(g[:], g[:], gps[:])
                    ga = wp.tile([128, GC, BL], f32, tag="ga")
                    nc.scalar.activation(ga[:, 0:12, :], g[:, 0:12, :], AF.Sigmoid)
                    nc.scalar.activation(ga[:, 12:16, :], g[:, 12:16, :], AF.Tanh)
                    tn = min(t + 1, T - 1)
                    odh = wp.tile([128, HC, BL], f32, tag="odh")
                    nc.vector.tensor_copy(odh[:], dht[:, :, :, tn])
                    nc.vector.tensor_mul(odh[:], ga[:, 8:12, :], odh[:])
                    tmp = wp.tile([128, HC, BL], f32, tag="tmp")
                    nc.vector.tensor_mul(tmp[:], ga[:, 0:4, :], ga[:, 12:16, :])
                    nc.vector.tensor_mul(c_st[:], c_st[:], ga[:, 4:8, :])
                    nc.vector.tensor_add(c_st[:], c_st[:], tmp[:])
                    th = wp.tile([128, HC, BL], f32, tag="th")
                    nc.scalar.activation(th[:], c_st[:], AF.Tanh)
                    nc.vector.tensor_mul(hst[:, :, :, s], ga[:, 8:12, :], th[:])
                    nc.vector.tensor_mul(hbf[:], odh[:], th[:])
                for hc in range(HC):
                    nc.sync.dma_start(
                        out_ap[hc][:, :, blk * SS:(blk + 1) * SS],
                        hst[:, hc, :, :])

    nc.compile()
    return nc


def _prep_weights(X_mean, Wi, bi, Wf, bf, Wo, bo, Wc, bc,
                  gx_w, gx_b, gh_W, gh_b):
    f16 = np.float16
    f32 = np.float32
    Wfull = np.concatenate([Wi, Wf, Wo, Wc], axis=1).astype(f32)   # [1536, 2048]
    bfull = np.concatenate([bi, bf, bo, bc]).astype(f32)           # [2048]
    pack = np.zeros((WROWS, G), f16)
    pack[0:512] = Wfull[0:512]          # Wg: xt rows
    pack[512:1024] = Wfull[1024:1536]   # Wg: m rows
    pack[1024:1536] = Wfull[512:1024]   # Wh
    # ghW packed [128, 4*512]: row p, col kc*512+j  <->  gh_W[kc*128+p, j]
    pack[1536:1664] = (gh_W.astype(f32).reshape(DC, 128, D)
                       .transpose(1, 0, 2).reshape(128, DC * D))
    # xmT packed [128, 4*256]: row p, col kc*256+t  <->  X_mean[t, kc*128+p]
    pack[1664:1792, 0:DC * T] = (X_mean.astype(f32).T.reshape(DC, 128, T)
                                 .transpose(1, 0, 2).reshape(128, DC * T))
    pvec = np.zeros((128, 28), f32)
    pvec[:, 0:4] = gx_w.astype(f32).reshape(4, 128).T
    pvec[:, 4:8] = gx_b.astype(f32).reshape(4, 128).T
    pvec[:, 8:12] = gh_b.astype(f32).reshape(4, 128).T
    pvec[:, 12:28] = bfull.reshape(16, 128).T
    return pack, pvec


def kernel(**inputs):
    from concourse.bass_utils import run_bass_kernel_spmd

    if "nc" not in _CACHE:
        _CACHE["nc"] = _build()
    nc = _CACHE["nc"]

    x = inputs["x"]
    pack, pvec = _prep_weights(
        inputs["X_mean"], inputs["Wi"], inputs["bi"], inputs["Wf"],
        inputs["bf"], inputs["Wo"], inputs["bo"], inputs["Wc"], inputs["bc"],
        inputs["gx_w"], inputs["gx_b"], inputs["gh_W"], inputs["gh_b"])

    in_maps = []
    for c in range(NCORES):
        in_maps.append({
            "xc": np.ascontiguousarray(x[c * BL:(c + 1) * BL]).astype(np.float16),
            "wpart": pack[c * WSH:(c + 1) * WSH],
            "pvec": pvec,
        })

    res = run_bass_kernel_spmd(
        nc, in_maps, core_ids=list(range(NCORES)),
        trace=bool(int(os.environ.get("LG_TRACE", "0"))))
    _CACHE["last_result"] = res

    out = np.empty((B, T, H), np.float32)
    for c in range(NCORES):
        hT = res.results[c]["h_out"]                  # [4, 128, 8, 256] f16
        out[c * BL:(c + 1) * BL] = (hT.transpose(2, 3, 0, 1)
                                    .reshape(BL, T, H).astype(np.float32))
    return out


# revision 4
# speedup vs baseline: 1.1294x; 1.1294x over previous
"""LGnet (LSTM-style gated recurrent net) Trainium2 Bass kernel.

B=64, T=256, D=512, H=512, COMB=1536. Data-parallel over batch across 8
NeuronCores (B_local=8 per core).

The axon tunnel to the cores moves ~38 MB/s H2D / ~22 MB/s D2H, so wall
time is transfer-bound, not compute-bound. Design, per call:
  - x ships per-core sliced, fp16, NATURAL layout ([8,4,256,512]); the
    feature-major transposes happen on device via XBAR DMA transpose.
  - the ~6.3M params ship fp16 ONCE in total: each core gets 1/8th
    (wpart) and an on-device AllGather reassembles the full pack.
  - output returns fp16 [4,128,8,256] per core.

Math decomposition (identical to the fp32 reference):
  comb @ W = xt' @ W[0:512] + h' @ W[512:1024] + m @ W[1024:1536].
  Only the h' term is recurrent; delta_x, delta_h, xt', and the
  non-recurrent gate preactivations "a" are precomputed for all (t,b)
  with large matmuls; the sequential phase does per-step [8,512] @
  [512,2048] fp16 matmuls against resident weights.

Per-core layouts (fp16 unless noted), free index r = b*256 + t:
  xtpT/mT : [128, 4, 8, 256]    x^T residents  [d%128, d//128, b, t]
  dht     : [128, 4, 8, 256]    delta_h^T
  aT      : [128, 16, 8, 256]   gate preactivations (i|f|o|c x4 chunks)
  wg      : [128, 8, 2048]      non-recurrent gate weights (xt rows then m rows)
  wh      : [128, 4, 2048]      recurrent rows 512:1024
  ghw     : [128, 4, 512]       gamma_h weight
  pvec    : [128, 28] f32       gxw(4) gxb(4) ghb(4) gate biases(16)
  h_out   : [4, 128, 8, 256]    h^T staged output [hc, p, b, t]
"""

import os
import numpy as np

B, T, D, H = 64, 256, 512, 512
NCORES = 8
BL = B // NCORES          # 8 per-core batch
DC, HC, GC = 4, 4, 16     # 128-chunks of D, H, and 4*H gate cols
G = 4 * H                 # 2048 gate columns
SS = 16                   # recurrence steps per output-staging block
WROWS = 1792              # weight pack rows (Wg 1024 + Wh 512 + ghW 128 + xm 128)
WSH = WROWS // NCORES     # 224 rows per core

_CACHE = {}


def _build():
    import concourse.bacc as bacc
    import concourse.tile as tile
    import concourse.mybir as mybir

    f32 = mybir.dt.float32
    f16 = mybir.dt.float16
    AF = mybir.ActivationFunctionType

    nc = bacc.Bacc("TRN2", target_bir_lowering=False, debug=False,
                   num_devices=NCORES)

    xc = nc.dram_tensor("xc", [BL, 4, T, D], f16, kind="ExternalInput").ap()
    wpart = nc.dram_tensor("wpart", [WSH, G], f16, kind="ExternalInput").ap()
    pv_in = nc.dram_tensor("pvec", [128, 28], f32, kind="ExternalInput").ap()
    out_ap = nc.dram_tensor("h_out", [HC, 128, BL, T], f16,
                            kind="ExternalOutput").ap()

    with tile.TileContext(nc) as tc:
        with (
            tc.tile_pool(name="const", bufs=1) as cp,
            tc.tile_pool(name="work", bufs=2) as wp,
            tc.tile_pool(name="xtr", bufs=2) as xp,
            tc.tile_pool(name="stage", bufs=2) as stp,
            tc.tile_pool(name="psA", bufs=2, space="PSUM") as ppa,
            tc.tile_pool(name="psB", bufs=2, space="PSUM") as ppb,
            tc.tile_pool(name="psR", bufs=2, space="PSUM") as ppr,
            tc.tile_pool(name="dram", bufs=1, space="DRAM") as dp,
        ):
            # ---- gather the weight pack from all cores ----
            # (collectives cannot read IO tensors -> bounce through internal DRAM)
            wbounce = dp.tile([WSH, G], f16)
            nc.sync.dma_start(wbounce[:], wpart[:])
            wfull = dp.tile([WROWS, G], f16)
            nc.gpsimd.collective_compute(
                "AllGather",
                mybir.AluOpType.bypass,
                replica_groups=[list(range(NCORES))],
                ins=[wbounce[:]],
                outs=[wfull[:]],
            )

            pvec = cp.tile([128, 28], f32)
            nc.sync.dma_start(pvec[:], pv_in[:])
            wg = cp.tile([128, 8, G], f16)
            for kc in range(8):
                nc.sync.dma_start(wg[:, kc, :], wfull[kc * 128:(kc + 1) * 128, :])
            wh = cp.tile([128, HC, G], f16)
            for kc in range(HC):
                nc.sync.dma_start(
                    wh[:, kc, :], wfull[1024 + kc * 128:1024 + (kc + 1) * 128, :])
            ghw = cp.tile([128, DC, D], f16)
            nc.sync.dma_start(ghw[:, :, :], wfull[1536:1664, :])
            xm = cp.tile([128, DC, T], f16)
            nc.sync.dma_start(xm[:, :, :], wfull[1664:1792, 0:DC * T])

            # ---- residents ----
            xtp = cp.tile([128, DC, BL, T], f16)   # xt^T, becomes xt'
            mT = cp.tile([128, DC, BL, T], f16)    # m^T
            dht = cp.tile([128, HC, BL, T], f16)   # delta_h^T
            aT = cp.tile([128, GC, BL, T], f16)    # gate preactivations

            # ---- phase 1 (per batch row): transpose + delta_h + delta_x + xt' ----
            for b in range(BL):
                dlT = xp.tile([128, DC, T], f16, tag="dlT")
                xlT = xp.tile([128, DC, T], f16, tag="xlT")
                nc.sync.dma_start_transpose(xtp[:, :, b, :], xc[b, 0])
                nc.sync.dma_start_transpose(xlT[:], xc[b, 1])
                nc.sync.dma_start_transpose(mT[:, :, b, :], xc[b, 2])
                nc.sync.dma_start_transpose(dlT[:], xc[b, 3])

                # delta_h = exp(-relu(dl @ ghW + ghb))
                for mt in range(HC):
                    ps = ppa.tile([128, T], f32, tag="dhps")
                    for kc in range(DC):
                        nc.tensor.matmul(
                            ps[:],
                            ghw[:, kc, mt * 128:(mt + 1) * 128],
                            dlT[:, kc, :],
                            start=(kc == 0), stop=(kc == DC - 1))
                    t1 = wp.tile([128, T], f32, tag="dht1")
                    nc.scalar.activation(t1[:], ps[:], AF.Relu,
                                         bias=pvec[:, 8 + mt:9 + mt], scale=1.0)
                    nc.scalar.activation(dht[:, mt, b, :], t1[:], AF.Exp,
                                         scale=-1.0)

                # delta_x = exp(-relu(dl * gxw + gxb)), in place into dlT
                for kc in range(DC):
                    t2 = wp.tile([128, T], f32, tag="dxt")
                    nc.scalar.activation(t2[:], dlT[:, kc, :], AF.Relu,
                                         bias=pvec[:, 4 + kc:5 + kc],
                                         scale=pvec[:, kc:kc + 1])
                    nc.scalar.activation(dlT[:, kc, :], t2[:], AF.Exp,
                                         scale=-1.0)

                # xt' = m*(xt - inner) + inner;  inner = dx*(xl - xm) + xm
                for kc in range(DC):
                    s1 = wp.tile([128, T], f16, tag="s1")
                    nc.vector.tensor_sub(s1[:], xlT[:, kc, :], xm[:, kc, :])
                    nc.vector.tensor_mul(s1[:], dlT[:, kc, :], s1[:])
                    nc.vector.tensor_add(s1[:], s1[:], xm[:, kc, :])
                    s2 = wp.tile([128, T], f16, tag="s2")
                    nc.vector.tensor_sub(s2[:], xtp[:, kc, b, :], s1[:])
                    nc.vector.tensor_mul(s2[:], mT[:, kc, b, :], s2[:])
                    nc.vector.tensor_add(xtp[:, kc, b, :], s2[:], s1[:])

            # ---- phase 2: a = xt'@Wx + m@Wm + bias ----
            for gc in range(GC):
                for nb in range(4):          # blocks of 2 batch rows = 512 cols
                    b0 = nb * 2
                    ps = ppb.tile([128, 2, T], f32, tag="aps")
                    for kc in range(8):
                        rhs = (xtp[:, kc, b0:b0 + 2, :] if kc < DC
                               else mT[:, kc - DC, b0:b0 + 2, :])
                        nc.tensor.matmul(ps[:], wg[:, kc, gc * 128:(gc + 1) * 128],
                                         rhs, start=(kc == 0), stop=(kc == 7))
                    nc.scalar.activation(aT[:, gc, b0:b0 + 2, :], ps[:],
                                         AF.Identity,
                                         bias=pvec[:, 12 + gc:13 + gc], scale=1.0)

            # ---- phase 3: recurrence ----
            c_st = cp.tile([128, HC, BL], f32)
            hbf = cp.tile([128, HC, BL], f16)
            nc.vector.memset(c_st[:], 0.0)
            nc.vector.memset(hbf[:], 0.0)

            for blk in range(T // SS):
                hst = stp.tile([128, HC, BL, SS], f16, tag="hst")
                for s in range(SS):
                    t = blk * SS + s
                    gps = ppr.tile([128, GC, BL], f32, tag="gps")
                    for gc in range(GC):
                        for kc in range(HC):
                            nc.tensor.matmul(
                                gps[:, gc, :],
                                wh[:, kc, gc * 128:(gc + 1) * 128],
                                hbf[:, kc, :],
                                start=(kc == 0), stop=(kc == HC - 1))
                    g = wp.tile([128, GC, BL], f32, tag="g")
                    nc.vector.tensor_copy(g[:], aT[:, :, :, t])
                    nc.vector.tensor_add(g[:], g[:], gps[:])
                    ga = wp.tile([128, GC, BL], f32, tag="ga")
                    nc.scalar.activation(ga[:, 0:12, :], g[:, 0:12, :], AF.Sigmoid)
                    nc.scalar.activation(ga[:, 12:16, :], g[:, 12:16, :], AF.Tanh)
                    tn = min(t + 1, T - 1)
                    odh = wp.tile([128, HC, BL], f32, tag="odh")
                    nc.vector.tensor_copy(odh[:], dht[:, :, :, tn])
                    nc.vector.tensor_mul(odh[:], ga[:, 8:12, :], odh[:])
                    tmp = wp.tile([128, HC, BL], f32, tag="tmp")
                    nc.vector.tensor_mul(tmp[:], ga[:, 0:4, :], ga[:, 12:16, :])
                    nc.vector.tensor_mul(c_st[:], c_st[:], ga[:, 4:8, :])
                    nc.vector.tensor_add(c_st[:], c_st[:], tmp[:])
                    th = wp.tile([128, HC, BL], f32, tag="th")
                    nc.scalar.activation(th[:], c_st[:], AF.Tanh)
                    nc.vector.tensor_mul(hst[:, :, :, s], ga[:, 8:12, :], th[:])
                    nc.vector.tensor_mul(hbf[:], odh[:], th[:])
                for hc in range(HC):
                    nc.sync.dma_start(
                        out_ap[hc][:, :, blk * SS:(blk + 1) * SS],
                        hst[:, hc, :, :])

    nc.compile()
    return nc


def _prep_weights(X_mean, Wi, bi, Wf, bf, Wo, bo, Wc, bc,
                  gx_w, gx_b, gh_W, gh_b):
    f16 = np.float16
    f32 = np.float32
    Wfull = np.concatenate([Wi, Wf, Wo, Wc], axis=1).astype(f32)   # [1536, 2048]
    bfull = np.concatenate([bi, bf, bo, bc]).astype(f32)           # [2048]
    pack = np.zeros((WROWS, G), f16)
    pack[0:512] = Wfull[0:512]          # Wg: xt rows
    pack[512:1024] = Wfull[1024:1536]   # Wg: m rows
    pack[1024:1536] = Wfull[512:1024]   # Wh
    # ghW packed [128, 4*512]: row p, col kc*512+j  <->  gh_W[kc*128+p, j]
    pack[1536:1664] = (gh_W.astype(f32).reshape(DC, 128, D)
                       .transpose(1, 0, 2).reshape(128, DC * D))
    # xmT packed [128, 4*256]: row p, col kc*256+t  <->  X_mean[t, kc*128+p]
    pack[1664:1792, 0:DC * T] = (X_mean.astype(f32).T.reshape(DC, 128, T)
                                 .transpose(1, 0, 2).reshape(128, DC * T))
    pvec = np.zeros((128, 28), f32)
    pvec[:, 0:4] = gx_w.astype(f32).reshape(4, 128).T
    pvec[:, 4:8] = gx_b.astype(f32).reshape(4, 128).T
    pvec[:, 8:12] = gh_b.astype(f32).reshape(4, 128).T
    pvec[:, 12:28] = bfull.reshape(16, 128).T
    return pack, pvec


def kernel(**inputs):
    import time
    from concourse.bass_utils import run_bass_kernel_spmd

    dbg = bool(int(os.environ.get("LG_TIME", "0")))
    t0 = time.time()
    if "nc" not in _CACHE:
        _CACHE["nc"] = _build()
    nc = _CACHE["nc"]

    t1 = time.time()
    x = inputs["x"]
    pack, pvec = _prep_weights(
        inputs["X_mean"], inputs["Wi"], inputs["bi"], inputs["Wf"],
        inputs["bf"], inputs["Wo"], inputs["bo"], inputs["Wc"], inputs["bc"],
        inputs["gx_w"], inputs["gx_b"], inputs["gh_W"], inputs["gh_b"])

    in_maps = []
    for c in range(NCORES):
        in_maps.append({
            "xc": x[c * BL:(c + 1) * BL].astype(np.float16),
            "wpart": pack[c * WSH:(c + 1) * WSH],
            "pvec": pvec,
        })

    t2 = time.time()
    res = run_bass_kernel_spmd(
        nc, in_maps, core_ids=list(range(NCORES)),
        trace=bool(int(os.environ.get("LG_TRACE", "0"))))
    _CACHE["last_result"] = res

    t3 = time.time()
    out = np.empty((B, T, H), np.float32)
    for c in range(NCORES):
        hT = res.results[c]["h_out"]                  # [4, 128, 8, 256] f16
        ov = out[c * BL:(c + 1) * BL].reshape(BL, T, HC, 128)
        ov[:] = hT.transpose(2, 3, 0, 1)
    t4 = time.time()
    if dbg:
        print(f"[kernel] build {t1 - t0:.3f}s prep {t2 - t1:.3f}s "
              f"run {t3 - t2:.3f}s asm {t4 - t3:.3f}s")
    return out


# revision 6
# speedup vs baseline: 1.1460x; 1.0148x over previous
"""LGnet (LSTM-style gated recurrent net) Trainium2 Bass kernel.

B=64, T=256, D=512, H=512, COMB=1536. Data-parallel over batch across 8
NeuronCores (B_local=8 per core).

The axon tunnel to the cores moves ~38 MB/s H2D / ~22 MB/s D2H, so wall
time is transfer-bound, not compute-bound. Design, per call:
  - x ships per-core sliced, fp16, NATURAL layout ([8,4,256,512]); the
    feature-major transposes happen on device via XBAR DMA transpose.
  - the ~6.3M params ship fp16 ONCE in total: each core gets 1/8th
    (wpart) and an on-device AllGather reassembles the full pack.
  - output returns fp16 [4,128,8,256] per core.

Math decomposition (identical to the fp32 reference):
  comb @ W = xt' @ W[0:512] + h' @ W[512:1024] + m @ W[1024:1536].
  Only the h' term is recurrent; delta_x, delta_h, xt', and the
  non-recurrent gate preactivations "a" are precomputed for all (t,b)
  with large matmuls; the sequential phase does per-step [8,512] @
  [512,2048] fp16 matmuls against resident weights.

Per-core layouts (fp16 unless noted), free index r = b*256 + t:
  xtpT/mT : [128, 4, 8, 256]    x^T residents  [d%128, d//128, b, t]
  dht     : [128, 4, 8, 256]    delta_h^T
  aT      : [128, 16, 8, 256]   gate preactivations (i|f|o|c x4 chunks)
  wg      : [128, 8, 2048]      non-recurrent gate weights (xt rows then m rows)
  wh      : [128, 4, 2048]      recurrent rows 512:1024
  ghw     : [128, 4, 512]       gamma_h weight
  pvec    : [128, 28] f32       gxw(4) gxb(4) ghb(4) gate biases(16)
  h_out   : [4, 128, 8, 256]    h^T staged output [hc, p, b, t]
"""

import os
import numpy as np

B, T, D, H = 64, 256, 512, 512
NCORES = 8
BL = B // NCORES          # 8 per-core batch
DC, HC, GC = 4, 4, 16     # 128-chunks of D, H, and 4*H gate cols
G = 4 * H                 # 2048 gate columns
SS = 16                   # recurrence steps per output-staging block
WROWS = 1792              # weight pack rows (Wg 1024 + Wh 512 + ghW 128 + xm 128)
WSH = WROWS // NCORES     # 224 rows per core

_CACHE = {}


def _build():
    import concourse.bacc as bacc
    import concourse.tile as tile
    import concourse.mybir as mybir

    f32 = mybir.dt.float32
    f16 = mybir.dt.float16
    AF = mybir.ActivationFunctionType

    nc = bacc.Bacc("TRN2", target_bir_lowering=False, debug=False,
                   num_devices=NCORES)

    xc = nc.dram_tensor("xc", [BL, 4, T, D], f16, kind="ExternalInput").ap()
    wpart = nc.dram_tensor("wpart", [WSH, G], f16, kind="ExternalInput").ap()
    pv_in = nc.dram_tensor("pvec", [128, 28], f32, kind="ExternalInput").ap()
    out_ap = nc.dram_tensor("h_out", [HC, 128, BL, T], f16,
                            kind="ExternalOutput").ap()

    with tile.TileContext(nc) as tc:
        with (
            tc.tile_pool(name="const", bufs=1) as cp,
            tc.tile_pool(name="work", bufs=2) as wp,
            tc.tile_pool(name="xtr", bufs=2) as xp,
            tc.tile_pool(name="stage", bufs=2) as stp,
            tc.tile_pool(name="psA", bufs=2, space="PSUM") as ppa,
            tc.tile_pool(name="psB", bufs=2, space="PSUM") as ppb,
            tc.tile_pool(name="psR", bufs=2, space="PSUM") as ppr,
            tc.tile_pool(name="dram", bufs=1, space="DRAM") as dp,
        ):
            # ---- gather the weight pack from all cores ----
            # (collectives cannot read IO tensors -> bounce through internal DRAM)
            wbounce = dp.tile([WSH, G], f16)
            nc.sync.dma_start(wbounce[:], wpart[:])
            wfull = dp.tile([WROWS, G], f16)
            nc.gpsimd.collective_compute(
                "AllGather",
                mybir.AluOpType.bypass,
                replica_groups=[list(range(NCORES))],
                ins=[wbounce[:]],
                outs=[wfull[:]],
            )

            pvec = cp.tile([128, 28], f32)
            nc.sync.dma_start(pvec[:], pv_in[:])
            wg = cp.tile([128, 8, G], f16)
            for kc in range(8):
                nc.sync.dma_start(wg[:, kc, :], wfull[kc * 128:(kc + 1) * 128, :])
            wh = cp.tile([128, HC, G], f16)
            for kc in range(HC):
                nc.sync.dma_start(
                    wh[:, kc, :], wfull[1024 + kc * 128:1024 + (kc + 1) * 128, :])
            ghw = cp.tile([128, DC, D], f16)
            nc.sync.dma_start(ghw[:, :, :], wfull[1536:1664, :])
            xm = cp.tile([128, DC, T], f16)
            nc.sync.dma_start(xm[:, :, :], wfull[1664:1792, 0:DC * T])

            # ---- residents ----
            xtp = cp.tile([128, DC, BL, T], f16)   # xt^T, becomes xt'
            mT = cp.tile([128, DC, BL, T], f16)    # m^T
            dht = cp.tile([128, HC, BL, T], f16)   # delta_h^T
            aT = cp.tile([128, GC, BL, T], f16)    # gate preactivations

            # ---- phase 1 (per batch row): transpose + delta_h + delta_x + xt' ----
            for b in range(BL):
                dlT = xp.tile([128, DC, T], f16, tag="dlT")
                xlT = xp.tile([128, DC, T], f16, tag="xlT")
                nc.sync.dma_start_transpose(xtp[:, :, b, :], xc[b, 0])
                nc.sync.dma_start_transpose(xlT[:], xc[b, 1])
                nc.sync.dma_start_transpose(mT[:, :, b, :], xc[b, 2])
                nc.sync.dma_start_transpose(dlT[:], xc[b, 3])

                # delta_h = exp(-relu(dl @ ghW + ghb))
                for mt in range(HC):
                    ps = ppa.tile([128, T], f32, tag="dhps")
                    for kc in range(DC):
                        nc.tensor.matmul(
                            ps[:],
                            ghw[:, kc, mt * 128:(mt + 1) * 128],
                            dlT[:, kc, :],
                            start=(kc == 0), stop=(kc == DC - 1))
                    t1 = wp.tile([128, T], f32, tag="dht1")
                    nc.scalar.activation(t1[:], ps[:], AF.Relu,
                                         bias=pvec[:, 8 + mt:9 + mt], scale=1.0)
                    nc.scalar.activation(dht[:, mt, b, :], t1[:], AF.Exp,
                                         scale=-1.0)

                # delta_x = exp(-relu(dl * gxw + gxb)), in place into dlT
                for kc in range(DC):
                    t2 = wp.tile([128, T], f32, tag="dxt")
                    nc.scalar.activation(t2[:], dlT[:, kc, :], AF.Relu,
                                         bias=pvec[:, 4 + kc:5 + kc],
                                         scale=pvec[:, kc:kc + 1])
                    nc.scalar.activation(dlT[:, kc, :], t2[:], AF.Exp,
                                         scale=-1.0)

                # xt' = m*(xt - inner) + inner;  inner = dx*(xl - xm) + xm
                for kc in range(DC):
                    s1 = wp.tile([128, T], f16, tag="s1")
                    nc.vector.tensor_sub(s1[:], xlT[:, kc, :], xm[:, kc, :])
                    nc.vector.tensor_mul(s1[:], dlT[:, kc, :], s1[:])
                    nc.vector.tensor_add(s1[:], s1[:], xm[:, kc, :])
                    s2 = wp.tile([128, T], f16, tag="s2")
                    nc.vector.tensor_sub(s2[:], xtp[:, kc, b, :], s1[:])
                    nc.vector.tensor_mul(s2[:], mT[:, kc, b, :], s2[:])
                    nc.vector.tensor_add(xtp[:, kc, b, :], s2[:], s1[:])

            # ---- phase 2: a = xt'@Wx + m@Wm + bias ----
            for gc in range(GC):
                for nb in range(4):          # blocks of 2 batch rows = 512 cols
                    b0 = nb * 2
                    ps = ppb.tile([128, 2, T], f32, tag="aps")
                    for kc in range(8):
                        rhs = (xtp[:, kc, b0:b0 + 2, :] if kc < DC
                               else mT[:, kc - DC, b0:b0 + 2, :])
                        nc.tensor.matmul(ps[:], wg[:, kc, gc * 128:(gc + 1) * 128],
                                         rhs, start=(kc == 0), stop=(kc == 7))
                    nc.scalar.activation(aT[:, gc, b0:b0 + 2, :], ps[:],
                                         AF.Identity,
                                         bias=pvec[:, 12 + gc:13 + gc], scale=1.0)

            # ---- phase 3: recurrence ----
            c_st = cp.tile([128, HC, BL], f32)
            hbf = cp.tile([128, HC, BL], f16)
            nc.vector.memset(c_st[:], 0.0)
            nc.vector.memset(hbf[:], 0.0)

            for blk in range(T // SS):
                hst = stp.tile([128, HC, BL, SS], f16, tag="hst")
                for s in range(SS):
                    t = blk * SS + s
                    gps = ppr.tile([128, GC, BL], f32, tag="gps")
                    for gc in range(GC):
                        for kc in range(HC):
                            nc.tensor.matmul(
                                gps[:, gc, :],
                                wh[:, kc, gc * 128:(gc + 1) * 128],
                                hbf[:, kc, :],
                                start=(kc == 0), stop=(kc == HC - 1))
                    g = wp.tile([128, GC, BL], f32, tag="g")
                    nc.vector.tensor_copy(g[:], aT[:, :, :, t])
                    nc.vector.tensor_add(g[:], g[:], gps[:])
                    ga = wp.tile([128, GC, BL], f32, tag="ga")
                    nc.scalar.activation(ga[:, 0:12, :], g[:, 0:12, :], AF.Sigmoid)
                    nc.scalar.activation(ga[:, 12:16, :], g[:, 12:16, :], AF.Tanh)
                    tn = min(t + 1, T - 1)
                    odh = wp.tile([128, HC, BL], f32, tag="odh")
                    nc.vector.tensor_copy(odh[:], dht[:, :, :, tn])
                    nc.vector.tensor_mul(odh[:], ga[:, 8:12, :], odh[:])
                    tmp = wp.tile([128, HC, BL], f32, tag="tmp")
                    nc.vector.tensor_mul(tmp[:], ga[:, 0:4, :], ga[:, 12:16, :])
                    nc.vector.tensor_mul(c_st[:], c_st[:], ga[:, 4:8, :])
                    nc.vector.tensor_add(c_st[:], c_st[:], tmp[:])
                    th = wp.tile([128, HC, BL], f32, tag="th")
                    nc.scalar.activation(th[:], c_st[:], AF.Tanh)
                    nc.vector.tensor_mul(hst[:, :, :, s], ga[:, 8:12, :], th[:])
                    nc.vector.tensor_mul(hbf[:], odh[:], th[:])
                for hc in range(HC):
                    nc.sync.dma_start(
                        out_ap[hc][:, :, blk * SS:(blk + 1) * SS],
                        hst[:, hc, :, :])

    nc.compile()
    # The pjrt path re-serializes the (now frozen) module on every call's
    # retrace (~0.2s for this 38K-instruction BIR); memoize on our instance.
    _json = nc.to_json_bytes()
    nc.to_json_bytes = lambda: _json
    return nc


def _prep_weights(X_mean, Wi, bi, Wf, bf, Wo, bo, Wc, bc,
                  gx_w, gx_b, gh_W, gh_b):
    f16 = np.float16
    f32 = np.float32
    Wfull = np.concatenate([Wi, Wf, Wo, Wc], axis=1).astype(f32)   # [1536, 2048]
    bfull = np.concatenate([bi, bf, bo, bc]).astype(f32)           # [2048]
    pack = np.zeros((WROWS, G), f16)
    pack[0:512] = Wfull[0:512]          # Wg: xt rows
    pack[512:1024] = Wfull[1024:1536]   # Wg: m rows
    pack[1024:1536] = Wfull[512:1024]   # Wh
    # ghW packed [128, 4*512]: row p, col kc*512+j  <->  gh_W[kc*128+p, j]
    pack[1536:1664] = (gh_W.astype(f32).reshape(DC, 128, D)
                       .transpose(1, 0, 2).reshape(128, DC * D))
    # xmT packed [128, 4*256]: row p, col kc*256+t  <->  X_mean[t, kc*128+p]
    pack[1664:1792, 0:DC * T] = (X_mean.astype(f32).T.reshape(DC, 128, T)
                                 .transpose(1, 0, 2).reshape(128, DC * T))
    pvec = np.zeros((128, 28), f32)
    pvec[:, 0:4] = gx_w.astype(f32).reshape(4, 128).T
    pvec[:, 4:8] = gx_b.astype(f32).reshape(4, 128).T
    pvec[:, 8:12] = gh_b.astype(f32).reshape(4, 128).T
    pvec[:, 12:28] = bfull.reshape(16, 128).T
    return pack, pvec


def kernel(**inputs):
    import time
    from concourse.bass_utils import run_bass_kernel_spmd

    dbg = bool(int(os.environ.get("LG_TIME", "0")))
    t0 = time.time()
    if "nc" not in _CACHE:
        _CACHE["nc"] = _build()
    nc = _CACHE["nc"]

    t1 = time.time()
    inputs = {k: np.asarray(v) for k, v in inputs.items()}
    x = inputs["x"]
    pack, pvec = _prep_weights(
        inputs["X_mean"], inputs["Wi"], inputs["bi"], inputs["Wf"],
        inputs["bf"], inputs["Wo"], inputs["bo"], inputs["Wc"], inputs["bc"],
        inputs["gx_w"], inputs["gx_b"], inputs["gh_W"], inputs["gh_b"])

    in_maps = []
    for c in range(NCORES):
        in_maps.append({
            "xc": x[c * BL:(c + 1) * BL].astype(np.float16),
            "wpart": pack[c * WSH:(c + 1) * WSH],
            "pvec": pvec,
        })

    t2 = time.time()
    res = run_bass_kernel_spmd(
        nc, in_maps, core_ids=list(range(NCORES)),
        trace=bool(int(os.environ.get("LG_TRACE", "0"))))
    _CACHE["last_result"] = res

    t3 = time.time()
    out = np.empty((B, T, H), np.float32)
    for c in range(NCORES):
        hT = res.results[c]["h_out"]                  # [4, 128, 8, 256] f16
        ov = out[c * BL:(c + 1) * BL].reshape(BL, T, HC, 128)
        ov[:] = hT.transpose(2, 3, 0, 1)
    t4 = time.time()
    if dbg:
        print(f"[kernel] build {t1 - t0:.3f}s prep {t2 - t1:.3f}s "
              f"run {t3 - t2:.3f}s asm {t4 - t3:.3f}s")
    return out


# revision 10
# speedup vs baseline: 1.6601x; 1.4486x over previous
"""LGnet (LSTM-style gated recurrent net) Trainium2 Bass kernel.

B=64, T=256, D=512, H=512, COMB=1536. Data-parallel over batch across 8
NeuronCores (B_local=8 per core).

The axon tunnel to the cores moves ~38 MB/s H2D / ~22 MB/s D2H, so wall
time is transfer-bound, not compute-bound. Design, per call:
  - x ships per-core sliced, fp16, NATURAL layout ([8,4,256,512]); the
    feature-major transposes happen on device via XBAR DMA transpose.
  - the ~6.3M params ship fp16 ONCE in total: each core gets 1/8th
    (wpart) and an on-device AllGather reassembles the full pack.
  - output returns fp16 [4,128,8,256] per core.

Math decomposition (identical to the fp32 reference):
  comb @ W = xt' @ W[0:512] + h' @ W[512:1024] + m @ W[1024:1536].
  Only the h' term is recurrent; delta_x, delta_h, xt', and the
  non-recurrent gate preactivations "a" are precomputed for all (t,b)
  with large matmuls; the sequential phase does per-step [8,512] @
  [512,2048] fp16 matmuls against resident weights.

Per-core layouts (fp16 unless noted), free index r = b*256 + t:
  xtpT/mT : [128, 4, 8, 256]    x^T residents  [d%128, d//128, b, t]
  dht     : [128, 4, 8, 256]    delta_h^T
  aT      : [128, 16, 8, 256]   gate preactivations (i|f|o|c x4 chunks)
  wg      : [128, 8, 2048]      non-recurrent gate weights (xt rows then m rows)
  wh      : [128, 4, 2048]      recurrent rows 512:1024
  ghw     : [128, 4, 512]       gamma_h weight
  pvec    : [128, 28] f32       gxw(4) gxb(4) ghb(4) gate biases(16)
  h_out   : [4, 128, 8, 256]    h^T staged output [hc, p, b, t]
"""

import os
import numpy as np

B, T, D, H = 64, 256, 512, 512
NCORES = 8
BL = B // NCORES          # 8 per-core batch
DC, HC, GC = 4, 4, 16     # 128-chunks of D, H, and 4*H gate cols
G = 4 * H                 # 2048 gate columns
SS = 16                   # recurrence steps per output-staging block
WROWS = 1792              # weight pack rows (Wg 1024 + Wh 512 + ghW 128 + xm 128)
WSH = WROWS // NCORES     # 224 rows per core

_CACHE = {}


def _build():
    import concourse.bacc as bacc
    import concourse.tile as tile
    import concourse.mybir as mybir

    f32 = mybir.dt.float32
    f16 = mybir.dt.float16
    AF = mybir.ActivationFunctionType

    nc = bacc.Bacc("TRN2", target_bir_lowering=False, debug=False,
                   num_devices=NCORES)

    xc = nc.dram_tensor("xc", [BL, 4, T, D], f16, kind="ExternalInput").ap()
    wpart = nc.dram_tensor("wpart", [WSH, G], f16, kind="ExternalInput").ap()
    pv_in = nc.dram_tensor("pvec", [128, 28], f32, kind="ExternalInput").ap()
    out_ap = nc.dram_tensor("h_out", [HC, 128, BL, T], f16,
                            kind="ExternalOutput").ap()

    with tile.TileContext(nc) as tc:
        with (
            tc.tile_pool(name="const", bufs=1) as cp,
            tc.tile_pool(name="work", bufs=2) as wp,
            tc.tile_pool(name="xtr", bufs=2) as xp,
            tc.tile_pool(name="stage", bufs=2) as stp,
            tc.tile_pool(name="psA", bufs=2, space="PSUM") as ppa,
            tc.tile_pool(name="psB", bufs=2, space="PSUM") as ppb,
            tc.tile_pool(name="psR", bufs=2, space="PSUM") as ppr,
            tc.tile_pool(name="dram", bufs=1, space="DRAM") as dp,
        ):
            # ---- gather the weight pack from all cores ----
            # (collectives cannot read IO tensors -> bounce through internal DRAM)
            wbounce = dp.tile([WSH, G], f16)
            nc.sync.dma_start(wbounce[:], wpart[:])
            wfull = dp.tile([WROWS, G], f16)
            nc.gpsimd.collective_compute(
                "AllGather",
                mybir.AluOpType.bypass,
                replica_groups=[list(range(NCORES))],
                ins=[wbounce[:]],
                outs=[wfull[:]],
            )

            pvec = cp.tile([128, 28], f32)
            nc.sync.dma_start(pvec[:], pv_in[:])
            wg = cp.tile([128, 8, G], f16)
            for kc in range(8):
                nc.sync.dma_start(wg[:, kc, :], wfull[kc * 128:(kc + 1) * 128, :])
            wh = cp.tile([128, HC, G], f16)
            for kc in range(HC):
                nc.sync.dma_start(
                    wh[:, kc, :], wfull[1024 + kc * 128:1024 + (kc + 1) * 128, :])
            ghw = cp.tile([128, DC, D], f16)
            nc.sync.dma_start(ghw[:, :, :], wfull[1536:1664, :])
            xm = cp.tile([128, DC, T], f16)
            nc.sync.dma_start(xm[:, :, :], wfull[1664:1792, 0:DC * T])

            # ---- residents ----
            xtp = cp.tile([128, DC, BL, T], f16)   # xt^T, becomes xt'
            mT = cp.tile([128, DC, BL, T], f16)    # m^T
            # t-major so the For_i recurrence can stage a block with one
            # dynamic-offset contiguous SBUF->SBUF DMA
            dht = cp.tile([128, T + SS, HC, BL], f16)   # delta_h^T
            aT = cp.tile([128, T, GC, BL], f16)         # gate preactivations
            nc.vector.memset(dht[:, T:, :, :], 0.0)

            # ---- phase 1 (per batch row): transpose + delta_h + delta_x + xt' ----
            for b in range(BL):
                dlT = xp.tile([128, DC, T], f16, tag="dlT")
                xlT = xp.tile([128, DC, T], f16, tag="xlT")
                nc.sync.dma_start_transpose(xtp[:, :, b, :], xc[b, 0])
                nc.sync.dma_start_transpose(xlT[:], xc[b, 1])
                nc.sync.dma_start_transpose(mT[:, :, b, :], xc[b, 2])
                nc.sync.dma_start_transpose(dlT[:], xc[b, 3])

                # delta_h = exp(-relu(dl @ ghW + ghb))
                for mt in range(HC):
                    ps = ppa.tile([128, T], f32, tag="dhps")
                    for kc in range(DC):
                        nc.tensor.matmul(
                            ps[:],
                            ghw[:, kc, mt * 128:(mt + 1) * 128],
                            dlT[:, kc, :],
                            start=(kc == 0), stop=(kc == DC - 1))
                    t1 = wp.tile([128, T], f32, tag="dht1")
                    nc.scalar.activation(t1[:], ps[:], AF.Relu,
                                         bias=pvec[:, 8 + mt:9 + mt], scale=1.0)
                    nc.scalar.activation(dht[:, 0:T, mt, b], t1[:], AF.Exp,
                                         scale=-1.0)

                # delta_x = exp(-relu(dl * gxw + gxb)), in place into dlT
                for kc in range(DC):
                    t2 = wp.tile([128, T], f32, tag="dxt")
                    nc.scalar.activation(t2[:], dlT[:, kc, :], AF.Relu,
                                         bias=pvec[:, 4 + kc:5 + kc],
                                         scale=pvec[:, kc:kc + 1])
                    nc.scalar.activation(dlT[:, kc, :], t2[:], AF.Exp,
                                         scale=-1.0)

                # xt' = m*(xt - inner) + inner;  inner = dx*(xl - xm) + xm
                for kc in range(DC):
                    s1 = wp.tile([128, T], f16, tag="s1")
                    nc.vector.tensor_sub(s1[:], xlT[:, kc, :], xm[:, kc, :])
                    nc.vector.tensor_mul(s1[:], dlT[:, kc, :], s1[:])
                    nc.vector.tensor_add(s1[:], s1[:], xm[:, kc, :])
                    s2 = wp.tile([128, T], f16, tag="s2")
                    nc.vector.tensor_sub(s2[:], xtp[:, kc, b, :], s1[:])
                    nc.vector.tensor_mul(s2[:], mT[:, kc, b, :], s2[:])
                    nc.vector.tensor_add(xtp[:, kc, b, :], s2[:], s1[:])

            # ---- phase 2: a = xt'@Wx + m@Wm + bias ----
            for gc in range(GC):
                for nb in range(4):          # blocks of 2 batch rows = 512 cols
                    b0 = nb * 2
                    ps = ppb.tile([128, 2, T], f32, tag="aps")
                    for kc in range(8):
                        rhs = (xtp[:, kc, b0:b0 + 2, :] if kc < DC
                               else mT[:, kc - DC, b0:b0 + 2, :])
                        nc.tensor.matmul(ps[:], wg[:, kc, gc * 128:(gc + 1) * 128],
                                         rhs, start=(kc == 0), stop=(kc == 7))
                    nc.scalar.activation(
                        aT[:, :, gc, b0:b0 + 2].rearrange("p t b -> p b t"),
                        ps[:], AF.Identity,
                        bias=pvec[:, 12 + gc:13 + gc], scale=1.0)

            # ---- phase 3: recurrence (hardware loop over SS-step blocks) ----
            from concourse.bass import ds

            c_st = cp.tile([128, HC, BL], f32)
            hbf = cp.tile([128, HC, BL], f16)
            nc.vector.memset(c_st[:], 0.0)
            nc.vector.memset(hbf[:], 0.0)

            with tc.For_i(0, T, SS) as t0:
                ab = stp.tile([128, SS, GC, BL], f16, tag="ab")
                nc.sync.dma_start(ab[:], aT[:, ds(t0, SS), :, :])
                db = stp.tile([128, SS, HC, BL], f16, tag="db")
                nc.sync.dma_start(db[:], dht[:, ds(t0 + 1, SS), :, :])
                hst = stp.tile([128, HC, BL, SS], f16, tag="hst")
                for s in range(SS):
                    gps = ppr.tile([128, GC, BL], f32, tag="gps")
                    for gc in range(GC):
                        for kc in range(HC):
                            nc.tensor.matmul(
                                gps[:, gc, :],
                                wh[:, kc, gc * 128:(gc + 1) * 128],
                                hbf[:, kc, :],
                                start=(kc == 0), stop=(kc == HC - 1))
                    g = wp.tile([128, GC, BL], f32, tag="g")
                    nc.vector.tensor_copy(g[:], ab[:, s, :, :])
                    nc.vector.tensor_add(g[:], g[:], gps[:])
                    ga = wp.tile([128, GC, BL], f32, tag="ga")
                    nc.scalar.activation(ga[:, 0:12, :], g[:, 0:12, :], AF.Sigmoid)
                    nc.scalar.activation(ga[:, 12:16, :], g[:, 12:16, :], AF.Tanh)
                    # db[s] = delta_h at t+1 (column T..T+SS-1 is zero padding;
                    # the hbf it feeds is never read again, so that's harmless)
                    odh = wp.tile([128, HC, BL], f32, tag="odh")
                    nc.vector.tensor_copy(odh[:], db[:, s, :, :])
                    nc.vector.tensor_mul(odh[:], ga[:, 8:12, :], odh[:])
                    tmp = wp.tile([128, HC, BL], f32, tag="tmp")
                    nc.vector.tensor_mul(tmp[:], ga[:, 0:4, :], ga[:, 12:16, :])
                    nc.vector.tensor_mul(c_st[:], c_st[:], ga[:, 4:8, :])
                    nc.vector.tensor_add(c_st[:], c_st[:], tmp[:])
                    th = wp.tile([128, HC, BL], f32, tag="th")
                    nc.scalar.activation(th[:], c_st[:], AF.Tanh)
                    nc.vector.tensor_mul(hst[:, :, :, s], ga[:, 8:12, :], th[:])
                    nc.vector.tensor_mul(hbf[:], odh[:], th[:])
                for hc in range(HC):
                    nc.sync.dma_start(out_ap[hc][:, :, ds(t0, SS)],
                                      hst[:, hc, :, :])

    nc.compile()
    # The pjrt path re-serializes the (now frozen) module on every call's
    # retrace (~0.2s for this 38K-instruction BIR); memoize on our instance.
    _json = nc.to_json_bytes()
    nc.to_json_bytes = lambda: _json
    return nc


def _prep_weights(X_mean, Wi, bi, Wf, bf, Wo, bo, Wc, bc,
                  gx_w, gx_b, gh_W, gh_b):
    f16 = np.float16
    f32 = np.float32
    Wfull = np.concatenate([Wi, Wf, Wo, Wc], axis=1).astype(f32)   # [1536, 2048]
    bfull = np.concatenate([bi, bf, bo, bc]).astype(f32)           # [2048]
    pack = np.zeros((WROWS, G), f16)
    pack[0:512] = Wfull[0:512]          # Wg: xt rows
    pack[512:1024] = Wfull[1024:1536]   # Wg: m rows
    pack[1024:1536] = Wfull[512:1024]   # Wh
    # ghW packed [128, 4*512]: row p, col kc*512+j  <->  gh_W[kc*128+p, j]
    pack[1536:1664] = (gh_W.astype(f32).reshape(DC, 128, D)
                       .transpose(1, 0, 2).reshape(128, DC * D))
    # xmT packed [128, 4*256]: row p, col kc*256+t  <->  X_mean[t, kc*128+p]
    pack[1664:1792, 0:DC * T] = (X_mean.astype(f32).T.reshape(DC, 128, T)
                                 .transpose(1, 0, 2).reshape(128, DC * T))
    pvec = np.zeros((128, 28), f32)
    pvec[:, 0:4] = gx_w.astype(f32).reshape(4, 128).T
    pvec[:, 4:8] = gx_b.astype(f32).reshape(4, 128).T
    pvec[:, 8:12] = gh_b.astype(f32).reshape(4, 128).T
    pvec[:, 12:28] = bfull.reshape(16, 128).T
    return pack, pvec


def kernel(**inputs):
    import time
    from concourse.bass_utils import run_bass_kernel_spmd

    dbg = bool(int(os.environ.get("LG_TIME", "0")))
    t0 = time.time()
    if "nc" not in _CACHE:
        _CACHE["nc"] = _build()
    nc = _CACHE["nc"]

    t1 = time.time()
    inputs = {k: np.asarray(v) for k, v in inputs.items()}
    x = inputs["x"]
    pack, pvec = _prep_weights(
        inputs["X_mean"], inputs["Wi"], inputs["bi"], inputs["Wf"],
        inputs["bf"], inputs["Wo"], inputs["bo"], inputs["Wc"], inputs["bc"],
        inputs["gx_w"], inputs["gx_b"], inputs["gh_W"], inputs["gh_b"])

    in_maps = []
    for c in range(NCORES):
        in_maps.append({
            "xc": x[c * BL:(c + 1) * BL].astype(np.float16),
            "wpart": pack[c * WSH:(c + 1) * WSH],
            "pvec": pvec,
        })

    t2 = time.time()
    res = run_bass_kernel_spmd(
        nc, in_maps, core_ids=list(range(NCORES)),
        trace=bool(int(os.environ.get("LG_TRACE", "0"))))
    _CACHE["last_result"] = res

    t3 = time.time()
    out = np.empty((B, T, H), np.float32)
    for c in range(NCORES):
        hT = res.results[c]["h_out"]                  # [4, 128, 8, 256] f16
        ov = out[c * BL:(c + 1) * BL].reshape(BL, T, HC, 128)
        ov[:] = hT.transpose(2, 3, 0, 1)
    t4 = time.time()
    if dbg:
        print(f"[kernel] build {t1 - t0:.3f}s prep {t2 - t1:.3f}s "
              f"run {t3 - t2:.3f}s asm {t4 - t3:.3f}s")
    return out
